# revision 9
# baseline (speedup 1.0000x reference)
import numpy as np
import ml_dtypes

import concourse.bacc as bacc
import concourse.tile as tile
from concourse import mybir

# NIMSCrossEntropyLoss: loss = [sum_px lse_c(p) - sum_px p[tgt]]/4, S=-1.
# v15: layout B (partition c*32+r), quarter streaming, all-fp8 matmul path:
#   P1 split into two 64KB pieces so the exp chain starts earlier;
#   W (fp8 ones-blocks) + cvec (fp8) packed into the first T transfer.
# Host ships (p-1): exp gives e^(p-1) which fits fp8 (max ~90 < 240), and
# the shift cancels exactly: sum lse - sum p_t == sum ln(s') - sum p'_t.
# PE: concurrent tile-positioned fp8 matmuls -> psumA/psumB (f32 exact-sum).
# DVE: 4 quarter mask-dot stts. ACT: 8*exp? no - 5 exps (512,512,1024x3) + 2 ln.

N_CORES = 8
P = 128
C = 4
N_BATCH = 4
FD = 4096
QD = FD // 4      # 1024
HQ = QD // 2      # 512

FP8 = mybir.dt.float8e4
BF16 = mybir.dt.bfloat16
F32 = mybir.dt.float32

_PATCHED = False


def _patch_act_tables():
    global _PATCHED
    if _PATCHED:
        return
    import concourse.hw_specs as hw_specs
    real = hw_specs.get_activation_tables
    Exp = mybir.ActivationFunctionType.Exp
    Ln = mybir.ActivationFunctionType.Ln

    def patched(arch):
        out = {}
        for name, fns in dict(real(arch)).items():
            if name != "natural_log_exp_and_others":
                fns = fns - {Exp, Ln}
            out[name] = fns
        return out

    bacc.get_activation_tables = patched
    _PATCHED = True


def build_nc(finalize=True):
    """out [P, 6] f32: cols 0..3 = quarter mask-dot accums,
    cols 4..5 = ln accums (halves, of ln(sum_c e^(p_c-1)))."""
    _patch_act_tables()
    nc = bacc.Bacc("TRN2", target_bir_lowering=False, debug=False)
    inP1a = nc.dram_tensor("inpP1a", (P, HQ), FP8, kind="ExternalInput").ap()
    inP1b = nc.dram_tensor("inpP1b", (P, HQ), FP8, kind="ExternalInput").ap()
    inP = [nc.dram_tensor(f"inpP{i}", (P, QD), FP8, kind="ExternalInput").ap()
           for i in range(1, 4)]
    inTWC = nc.dram_tensor("inpTWC", (P, 33 + QD), FP8, kind="ExternalInput").ap()
    inT = [nc.dram_tensor(f"inpT{i}", (P, QD), FP8, kind="ExternalInput").ap()
           for i in range(1, 4)]
    out = nc.dram_tensor("out", (P, 6), F32, kind="ExternalOutput").ap()

    Exp = mybir.ActivationFunctionType.Exp
    Ln = mybir.ActivationFunctionType.Ln

    with tile.TileContext(nc) as tc:
        with tc.tile_pool(name="w", bufs=1) as w, \
             tc.tile_pool(name="ps", bufs=1, space="PSUM") as ps:
            tP = [w.tile([P, QD], FP8, name=f"tP{i}") for i in range(4)]
            tTWC = w.tile([P, 33 + QD], FP8, name="tTWC")
            tT = [tTWC[:, 33:33 + QD]] + \
                 [w.tile([P, QD], FP8, name=f"tT{i}") for i in range(1, 4)]
            tW = tTWC[:, 0:32]
            tC = tTWC[:, 32:33]

            nc.sync.dma_start(out=tP[0][:, 0:HQ], in_=inP1a)
            nc.sync.dma_start(out=tP[0][:, HQ:QD], in_=inP1b)
            for i in range(1, 4):
                nc.sync.dma_start(out=tP[i], in_=inP[i - 1])
            nc.gpsimd.dma_start(out=tTWC, in_=inTWC)
            for i in range(1, 4):
                nc.gpsimd.dma_start(out=tT[i], in_=inT[i - 1])

            res = w.tile([P, 6], F32, name="res")
            e = [w.tile([P, QD], FP8, name=f"e{i}") for i in range(4)]
            psumA = ps.tile([P, 512], F32, name="psumA")
            psumB = ps.tile([P, 512], F32, name="psumB")

            # exp chain: first quarter in two halves (starts sooner)
            nc.scalar.activation(out=e[0][:, 0:HQ], in_=tP[0][:, 0:HQ], func=Exp)
            nc.scalar.activation(out=e[0][:, HQ:QD], in_=tP[0][:, HQ:QD], func=Exp)
            for i in range(1, 4):
                nc.scalar.activation(out=e[i], in_=tP[i], func=Exp)

            for h, pt in ((0, psumA), (2, psumB)):
                for j in range(4):
                    qi = h + j // 2
                    sl = (j % 2) * 512
                    nc.tensor.matmul(out=pt[j * 32:(j + 1) * 32, :],
                                     lhsT=tW, rhs=e[qi][:, sl:sl + 512],
                                     start=True, stop=True,
                                     tile_position=(0, j * 32))

            scr = w.tile([P, QD], BF16, name="scr")
            for i in range(4):
                nc.vector.scalar_tensor_tensor(
                    out=scr, in0=tT[i], scalar=tC, in1=tP[i],
                    op0=mybir.AluOpType.is_equal, op1=mybir.AluOpType.mult,
                    accum_out=res[:, i:i + 1],
                )

            lnout = w.tile([P, 512], BF16, name="lnout")
            nc.scalar.activation(out=lnout, in_=psumA, func=Ln,
                                 accum_out=res[:, 4:5])
            nc.scalar.activation(out=lnout, in_=psumB, func=Ln,
                                 accum_out=res[:, 5:6])

            nc.sync.dma_start(out=out, in_=res)
    if finalize:
        nc.finalize()
    return nc


_NC_CACHE = {}


def _get_nc():
    if "nc" not in _NC_CACHE:
        _NC_CACHE["nc"] = build_nc()
    return _NC_CACHE["nc"]


def prep_inputs(preds, targets):
    p = np.asarray(preds)[:, -1]
    t = np.asarray(targets)[:, -1]
    arr = np.transpose(p, (1, 0, 2, 3)).reshape(C, N_CORES, 32, FD)
    arr = (arr - 1.0).astype(ml_dtypes.float8_e4m3)
    tf = t.reshape(N_CORES, 32, FD).astype(ml_dtypes.float8_e4m3)
    WC = np.zeros((P, 33), dtype=ml_dtypes.float8_e4m3)
    for pp in range(P):
        WC[pp, pp % 32] = 1.0
        WC[pp, 32] = pp // 32
    maps = []
    for k in range(N_CORES):
        pb = arr[:, k].reshape(P, FD)
        trep = np.tile(tf[k], (4, 1))
        m = {
            "inpP1a": np.ascontiguousarray(pb[:, 0:HQ]),
            "inpP1b": np.ascontiguousarray(pb[:, HQ:QD]),
            "inpTWC": np.ascontiguousarray(
                np.concatenate([WC, trep[:, 0:QD]], axis=1)),
        }
        for i in range(1, 4):
            m[f"inpP{i}"] = np.ascontiguousarray(pb[:, i * QD:(i + 1) * QD])
            m[f"inpT{i}"] = np.ascontiguousarray(trep[:, i * QD:(i + 1) * QD])
        maps.append(m)
    return maps


def reduce_outputs(results):
    total = 0.0
    for d in results:
        o = d["out"].astype(np.float64)
        total += float(o[:, 4:6].sum() - o[:, 0:4].sum())
    return np.float32(total / N_BATCH)


def kernel(preds, targets, _trace=False, _trace_kwargs=None):
    from concourse.bass_utils import run_bass_kernel_spmd

    in_maps = prep_inputs(preds, targets)
    nc = _get_nc()
    r = run_bass_kernel_spmd(
        nc, in_maps, core_ids=list(range(N_CORES)),
        trace=_trace, **(_trace_kwargs or {}),
    )
    kernel.last_run = r
    return reduce_outputs(r.results)


kernel.last_run = None


# revision 10
# speedup vs baseline: 1.0727x; 1.0727x over previous
import numpy as np
import ml_dtypes

import concourse.bacc as bacc
import concourse.tile as tile
from concourse import mybir

# NIMSCrossEntropyLoss: loss = [sum_px lse_c(p) - sum_px p[tgt]]/4, S=-1.
# v17: layout B (partition c*32+r), quarter-granular streaming:
#   P1..P4 fp8 [128,1024] pred quarters   (sync/HWDGE, small first chunk)
#   T1..T4 fp8 [128,1024] target-replica  (gpsimd/SWDGE after cvec+W)
#   W bf16 [128,32] ones-blocks; cvec f32 [128,1] = p//32
# ACT: exp per quarter -> ln(psumA), ln(psumB).
# PE: 4 concurrent tile-positioned matmuls per half into psumA/psumB.
# DVE: 4 quarter mask-dot stts (is_equal vs cvec, mult, accum).

N_CORES = 8
P = 128
C = 4
N_BATCH = 4
FD = 4096
QD = FD // 4      # 1024 per quarter

FP8 = mybir.dt.float8e4
BF16 = mybir.dt.bfloat16
F32 = mybir.dt.float32

_PATCHED = False


def _patch_act_tables():
    global _PATCHED
    if _PATCHED:
        return
    import concourse.hw_specs as hw_specs
    real = hw_specs.get_activation_tables
    Exp = mybir.ActivationFunctionType.Exp
    Ln = mybir.ActivationFunctionType.Ln

    def patched(arch):
        out = {}
        for name, fns in dict(real(arch)).items():
            if name != "natural_log_exp_and_others":
                fns = fns - {Exp, Ln}
            out[name] = fns
        return out

    bacc.get_activation_tables = patched
    _PATCHED = True


def build_nc(finalize=True):
    """out [P, 6] f32: cols 0..3 = quarter mask-dot accums,
    cols 4..5 = ln accums (halves)."""
    _patch_act_tables()
    nc = bacc.Bacc("TRN2", target_bir_lowering=False, debug=False)
    inP0a = nc.dram_tensor("inpP0a", (P, QD // 2), FP8, kind="ExternalInput").ap()
    inP0b = nc.dram_tensor("inpP0b", (P, QD // 2), FP8, kind="ExternalInput").ap()
    inP = [nc.dram_tensor(f"inpP{i}", (P, QD), FP8, kind="ExternalInput").ap()
           for i in range(1, 4)]
    inT = [nc.dram_tensor(f"inpT{i}", (P, QD), FP8, kind="ExternalInput").ap()
           for i in range(4)]
    inpWC = nc.dram_tensor("inpWC", (P, 33), BF16, kind="ExternalInput").ap()
    out = nc.dram_tensor("out", (P, 7), F32, kind="ExternalOutput").ap()

    Exp = mybir.ActivationFunctionType.Exp
    Ln = mybir.ActivationFunctionType.Ln

    with tile.TileContext(nc) as tc:
        with tc.tile_pool(name="w", bufs=1) as w, \
             tc.tile_pool(name="ps", bufs=1, space="PSUM") as ps:
            tP0a = w.tile([P, QD // 2], FP8, name="tP0a")
            tP0b = w.tile([P, QD // 2], FP8, name="tP0b")
            tP = [None] + [w.tile([P, QD], FP8, name=f"tP{i}") for i in range(1, 4)]
            tT = [w.tile([P, QD], FP8, name=f"tT{i}") for i in range(4)]
            tWC = w.tile([P, 33], BF16, name="tWC")
            tW = tWC[:, 0:32]
            tC = tWC[:, 32:33]

            # 3rd queue (ACT-issued HWDGE) carries T2, issued before the
            # table load so the ACT sequencer cost is hidden.
            nc.scalar.dma_start(out=tT[1], in_=inT[1])
            nc.sync.dma_start(out=tP0a, in_=inP0a)
            nc.sync.dma_start(out=tP0b, in_=inP0b)
            for i in range(1, 4):
                nc.sync.dma_start(out=tP[i], in_=inP[i - 1])
            nc.gpsimd.dma_start(out=tWC, in_=inpWC)
            nc.gpsimd.dma_start(out=tT[0], in_=inT[0])
            nc.gpsimd.dma_start(out=tT[2], in_=inT[2])
            nc.gpsimd.dma_start(out=tT[3], in_=inT[3])

            res = w.tile([P, 7], F32, name="res")
            e = [w.tile([P, QD], BF16, name=f"e{i}") for i in range(4)]
            psumA = ps.tile([P, 512], F32, name="psumA")
            psumB = ps.tile([P, 512], F32, name="psumB")

            nc.scalar.activation(out=e[0][:, 0:512], in_=tP0a, func=Exp)
            nc.scalar.activation(out=e[0][:, 512:QD], in_=tP0b, func=Exp)
            for i in range(1, 4):
                nc.scalar.activation(out=e[i], in_=tP[i], func=Exp)

            # channel-sum matmuls: half A = quarters 0,1; half B = 2,3.
            # Each quarter contributes two 512-col col-group matmuls.
            for h, pt in ((0, psumA), (2, psumB)):
                for j in range(4):
                    qi = h + j // 2
                    sl = (j % 2) * 512
                    nc.tensor.matmul(out=pt[j * 32:(j + 1) * 32, :],
                                     lhsT=tW, rhs=e[qi][:, sl:sl + 512],
                                     start=True, stop=True,
                                     tile_position=(0, j * 32))

            scr = w.tile([P, QD], BF16, name="scr")

            def dot(tti, pin, col):
                nc.vector.scalar_tensor_tensor(
                    out=scr[:, 0:pin.shape[1]], in0=tti, scalar=tC, in1=pin,
                    op0=mybir.AluOpType.is_equal, op1=mybir.AluOpType.mult,
                    accum_out=res[:, col:col + 1],
                )

            dot(tT[0][:, 0:512], tP0a, 0)
            dot(tT[1], tP[1], 1)
            dot(tT[0][:, 512:QD], tP0b, 5)
            dot(tT[2], tP[2], 2)
            dot(tT[3], tP[3], 3)

            lnout = w.tile([P, 512], BF16, name="lnout")
            nc.scalar.activation(out=lnout, in_=psumA, func=Ln,
                                 accum_out=res[:, 4:5])
            nc.scalar.activation(out=lnout, in_=psumB, func=Ln,
                                 accum_out=res[:, 6:7])

            nc.sync.dma_start(out=out, in_=res)
    if finalize:
        nc.finalize()
    return nc


_NC_CACHE = {}


def _get_nc():
    if "nc" not in _NC_CACHE:
        _NC_CACHE["nc"] = build_nc()
    return _NC_CACHE["nc"]


def prep_inputs(preds, targets):
    p = np.asarray(preds)[:, -1]
    t = np.asarray(targets)[:, -1]
    arr = np.transpose(p, (1, 0, 2, 3)).reshape(C, N_CORES, 32, FD)
    arr = arr.astype(ml_dtypes.float8_e4m3)
    tf = t.reshape(N_CORES, 32, FD).astype(ml_dtypes.float8_e4m3)
    WC = np.zeros((P, 33), dtype=ml_dtypes.bfloat16)
    for pp in range(P):
        WC[pp, pp % 32] = 1.0
        WC[pp, 32] = pp // 32
    maps = []
    for k in range(N_CORES):
        pb = arr[:, k].reshape(P, FD)
        trep = np.tile(tf[k], (4, 1))
        m = {"inpWC": WC}
        m["inpP0a"] = np.ascontiguousarray(pb[:, 0:QD // 2])
        m["inpP0b"] = np.ascontiguousarray(pb[:, QD // 2:QD])
        for i in range(4):
            if i:
                m[f"inpP{i}"] = np.ascontiguousarray(pb[:, i * QD:(i + 1) * QD])
            m[f"inpT{i}"] = np.ascontiguousarray(trep[:, i * QD:(i + 1) * QD])
        maps.append(m)
    return maps


def reduce_outputs(results):
    total = 0.0
    for d in results:
        o = d["out"].astype(np.float64)
        total += float(o[:, 4:5].sum() + o[:, 6:7].sum()
                       - o[:, 0:4].sum() - o[:, 5:6].sum())
    return np.float32(total / N_BATCH)


def kernel(preds, targets, _trace=False, _trace_kwargs=None):
    from concourse.bass_utils import run_bass_kernel_spmd

    in_maps = prep_inputs(preds, targets)
    nc = _get_nc()
    r = run_bass_kernel_spmd(
        nc, in_maps, core_ids=list(range(N_CORES)),
        trace=_trace, **(_trace_kwargs or {}),
    )
    kernel.last_run = r
    return reduce_outputs(r.results)


kernel.last_run = None
